# revision 6
# baseline (speedup 1.0000x reference)
"""Trainium2 Bass kernel for DigitCapsuleLayer dynamic routing.

Strategy: data-parallel over batch (32 per core x 8 cores). The routing is
computed in a fully factored form that never materializes u_hat
[B,1152,10,16]:

  q[b,c,m,i] = sum_g  cij[(g,m),c] * u[b,(g,m),i]      (PE, block-diag cij)
  s[b,c,o]   = sum_mi W[m,c,o,i]   * q[b,c,m,i]        (PE, after a DVE
                                                        32x32 block transpose
                                                        moves i to partitions)
  v = squash(s)                                        (PE ones-trick + DVE/ACT)
  p[b,c,m,i] = sum_o  W[m,c,o,i]   * v[b,c,o]          (PE, block-diag v)
  a[r,c]     = sum_bi u[b,r,i]/B   * p[b,c,m,i]        (PE)
  AllReduce(a) across 8 cores; b_ij += a; cij = softmax(b_ij)

Indices: r = g*32+m (g<36, m<32), m = 2t+m_sub, i = h*4+i4 (h<2, i4<4).
Partition layouts:  P1 rows = m_sub*64+g (rows 36..63/100..127 zero-padded),
q/p rows = i4*32+b (per half/blk), s/v rows = i4*16+o (4x replicated).
All index algebra validated against the jax reference in proto.py.
"""

import os
import sys
import numpy as np

sys.path.insert(0, "/opt/trn_rl_repo")
sys.path.insert(0, "/opt/trn_rl_repo/concourse")

NC_CORES = 8
BL = 32           # batch per core
G, M32, C, O, I = 36, 32, 10, 16, 8
T16 = 16
F32 = None        # set after mybir import


# ----------------------------------------------------------------- host prep
def _host_prep(u, W):
    """u [256,1152,8] f32, W [32,10,16,8] f32 -> per-core input maps."""
    u = np.ascontiguousarray(u, np.float32)
    W = np.ascontiguousarray(W, np.float32)

    # u_qp [core, 128, t, h, 128]; row p = m_sub*64+g ; col = i4*32+b
    u8 = u.reshape(NC_CORES, BL, G, T16, 2, 2, 4)   # [n, b, g, t, ms, h, i4]
    perm = u8.transpose(0, 2, 4, 3, 5, 6, 1)        # [n, g, ms, t, h, i4, b]
    u_qp = np.zeros((NC_CORES, 128, T16, 2, 128), np.float32)
    u_qp_v = u_qp.reshape(NC_CORES, 128, T16, 2, 4, 32)
    for ms in range(2):
        u_qp_v[:, ms * 64:ms * 64 + G] = perm[:, :, ms]

    # u_a2 [core, k, 128, 36, 32]: row i4*32+b, col (g, m); prescaled by 1/256
    ua = u.reshape(NC_CORES, BL, G, M32, 2, 4)      # [n, b, g, m, k, i4]
    u_a2 = np.ascontiguousarray(
        ua.transpose(0, 4, 5, 1, 2, 3), np.float32
    ).reshape(NC_CORES, 2, 128, G, M32) * np.float32(1.0 / 256.0)

    # w_s4 [k, 128, c, 64]: row i4*32+m, col (c, rep*16+o) = W[m,c,o,k*4+i4]
    wi = W.transpose(3, 0, 1, 2)                    # [i, m, c, o]
    w_s4 = np.broadcast_to(
        wi.reshape(2, 4, M32, C, 1, O), (2, 4, M32, C, 4, O)
    ).reshape(2, 128, C, 64).astype(np.float32)
    w_s4 = np.ascontiguousarray(w_s4)

    # w_p2 [64, c, blk, m]: row i4*16+o = W[m, c, o, blk*4+i4]
    wp = W.transpose(3, 2, 1, 0)                    # [i, o, c, m]
    w_p2 = np.ascontiguousarray(
        wp.reshape(2, 4, O, C, M32).transpose(1, 2, 3, 0, 4), np.float32
    ).reshape(64, C, 2, M32)

    ones_bd = np.kron(np.eye(4, dtype=np.float32), np.ones((16, 16), np.float32))

    in_maps = []
    for c in range(NC_CORES):
        in_maps.append({
            "u_qp": u_qp[c],
            "u_a2": u_a2[c],
            "w_s4": w_s4,
            "w_p2": w_p2,
            "ones_bd": ones_bd,
        })
    return in_maps


# ------------------------------------------------------------- bass builder
def _build_nc():
    from contextlib import ExitStack
    import concourse.bacc as bacc
    import concourse.tile as tile
    from concourse import mybir

    f32 = mybir.dt.float32
    nc = bacc.Bacc("TRN2", target_bir_lowering=False, debug=False,
                   num_devices=NC_CORES)

    u_qp_p = nc.dram_tensor("u_qp", [128, T16, 2, 128], f32, kind="ExternalInput")
    u_a2_p = nc.dram_tensor("u_a2", [2, 128, G, M32], f32, kind="ExternalInput")
    w_s4_p = nc.dram_tensor("w_s4", [2, 128, C, 64], f32, kind="ExternalInput")
    w_p2_p = nc.dram_tensor("w_p2", [64, C, 2, M32], f32, kind="ExternalInput")
    ones_p = nc.dram_tensor("ones_bd", [64, 64], f32, kind="ExternalInput")
    v_out_p = nc.dram_tensor("v_out", [16, C, BL], f32, kind="ExternalOutput")

    rg = [list(range(NC_CORES))]

    with tile.TileContext(nc) as tc, ExitStack() as ctx:
        sb = ctx.enter_context(tc.tile_pool(name="sb", bufs=1))
        ps = ctx.enter_context(tc.tile_pool(name="ps", bufs=1, space="PSUM"))
        dr = ctx.enter_context(tc.tile_pool(name="dr", bufs=1, space="DRAM"))

        # --- persistent SBUF tiles ---
        u_qp = sb.tile([128, T16, 2, 128], f32, tag="u_qp")
        u_a2 = [sb.tile([128, G, M32], f32, tag=f"u_a2_{k}", name=f"u_a2_{k}") for k in range(2)]
        w_s4 = [sb.tile([128, C, 64], f32, tag=f"w_s4_{k}", name=f"w_s4_{k}") for k in range(2)]
        w_p2 = sb.tile([64, C, 2, M32], f32, tag="w_p2")
        ones = sb.tile([64, 64], f32, tag="ones")
        b_ij = sb.tile([128, T16, C], f32, tag="b_ij")
        cij_bd = sb.tile([128, T16, 2, C], f32, tag="cij_bd")
        v_bd = sb.tile([64, C, 4, BL], f32, tag="v_bd")

        nc.sync.dma_start(out=u_qp[:], in_=u_qp_p[:])
        for k in range(2):
            nc.sync.dma_start(out=u_a2[k][:], in_=u_a2_p[k])
            nc.sync.dma_start(out=w_s4[k][:], in_=w_s4_p[k])
        nc.sync.dma_start(out=w_p2[:], in_=w_p2_p[:])
        nc.sync.dma_start(out=ones[:], in_=ones_p[:])
        nc.vector.memset(b_ij[:], 0.0)
        nc.vector.memset(cij_bd[:], 0.0)
        nc.vector.memset(v_bd[:], 0.0)

        # --- PSUM tiles (persist across iterations) ---
        q_psum = [ps.tile([128, T16, 2, C], f32, tag=f"q_ps{h}", name=f"q_ps{h}") for h in range(2)]
        s_psum = ps.tile([64, C, BL], f32, tag="s_ps")
        mag_ps = ps.tile([64, C, BL], f32, tag="mag_ps")
        p_psum = [ps.tile([128, C, M32], f32, tag=f"p_ps{b}", name=f"p_ps{b}") for b in range(2)]
        a_psum = ps.tile([128, T16, C], f32, tag="a_ps")
        nc.vector.memset(a_psum[:], 0.0)

        for it in range(3):
            last = it == 2
            # ---- softmax over capsules (no max-shift; logits are bounded) ----
            eb = sb.tile([128, T16, C], f32, tag="eb")
            ssum = sb.tile([128, T16], f32, tag="ssum")
            rs = sb.tile([128, T16], f32, tag="rs")
            cij = sb.tile([128, T16, C], f32, tag="cij")
            nc.scalar.activation(eb[:], b_ij[:], mybir.ActivationFunctionType.Exp)
            nc.vector.tensor_reduce(ssum[:], eb[:], axis=mybir.AxisListType.X,
                                    op=mybir.AluOpType.add)
            nc.vector.reciprocal(rs[:], ssum[:])
            nc.vector.tensor_mul(cij[:], eb[:],
                                 rs[:, :, None].broadcast_to((128, T16, C)))
            for ms in range(2):
                nc.vector.tensor_copy(out=cij_bd[ms * 64:ms * 64 + G, :, ms, :],
                                      in_=cij[ms * 64:ms * 64 + G, :, :])

            # ---- q: per (t, h) matmul, K=128 over (m_sub, g) ----
            for h in range(2):
                for t in range(T16):
                    nc.tensor.matmul(q_psum[h][:, t], lhsT=u_qp[:, t, h, :],
                                     rhs=cij_bd[:, t, :, :], start=True, stop=True)
            # reorder copy PSUM->SBUF: col c*32 + t*2 + ms
            q_sb = [sb.tile([128, C, M32], f32, tag=f"q_sb{h}", name=f"q_sb{h}") for h in range(2)]
            q_T = [sb.tile([128, C, M32], f32, tag=f"q_T{h}", name=f"q_T{h}") for h in range(2)]
            for h in range(2):
                dst = q_sb[h][:].rearrange("p c (t ms) -> p t ms c", ms=2)
                nc.vector.tensor_copy(out=dst, in_=q_psum[h][:])
                nc.vector.transpose(out=q_T[h][:], in_=q_sb[h][:])

            # ---- s: per (c, k) accumulate; out rows = (i4, o) 4x-replicated ----
            for c10 in range(C):
                for k in range(2):
                    nc.tensor.matmul(s_psum[:, c10, :], lhsT=w_s4[k][:, c10, :],
                                     rhs=q_T[k][:, c10, :],
                                     start=(k == 0), stop=(k == 1))

            # ---- squash on [64=(i4,o), c, b] ----
            s2 = sb.tile([64, C, BL], f32, tag="s2")
            e1 = sb.tile([64, C, BL], f32, tag="e1")
            root = sb.tile([64, C, BL], f32, tag="root")
            den = sb.tile([64, C, BL], f32, tag="den")
            rcp = sb.tile([64, C, BL], f32, tag="rcp")
            fsc = sb.tile([64, C, BL], f32, tag="fsc")
            v_rep = sb.tile([64, C, BL], f32, tag="v_rep")
            nc.scalar.square(s2[:], s_psum[:])
            nc.tensor.matmul(mag_ps[:], lhsT=ones[:], rhs=s2[:], start=True, stop=True)
            nc.scalar.add(e1[:], mag_ps[:], 1.0)
            nc.scalar.sqrt(root[:], mag_ps[:])
            nc.vector.tensor_mul(den[:], e1[:], root[:])
            nc.vector.reciprocal(rcp[:], den[:])
            nc.vector.tensor_mul(fsc[:], mag_ps[:], rcp[:])
            nc.vector.tensor_mul(v_rep[:], s_psum[:], fsc[:])

            if last:
                nc.sync.dma_start(out=v_out_p[:], in_=v_rep[0:16, :, :])
                break

            # ---- v_bd diag + p ----
            for j in range(4):
                nc.sync.dma_start(out=v_bd[j * 16:(j + 1) * 16, :, j, :],
                                  in_=v_rep[j * 16:(j + 1) * 16, :, :])
            p_sb = [sb.tile([128, C, M32], f32, tag=f"p_sb{b}", name=f"p_sb{b}") for b in range(2)]
            for blk in range(2):
                for c10 in range(C):
                    nc.tensor.matmul(p_psum[blk][:, c10, :],
                                     lhsT=v_bd[:, c10, :, :],
                                     rhs=w_p2[:, c10, blk, :],
                                     start=True, stop=True)
                nc.vector.tensor_copy(out=p_sb[blk][:], in_=p_psum[blk][:])

            # ---- a: per (m, k) accumulate into [ (m%2)*64+g , m//2, c ] ----
            for m in range(M32):
                t, ms = m // 2, m % 2
                for k in range(2):
                    nc.tensor.matmul(a_psum[ms * 64:ms * 64 + G, t, :],
                                     lhsT=u_a2[k][:, :, m],
                                     rhs=p_sb[k][:, :, m],
                                     start=(k == 0), stop=(k == 1))
            a_sb = sb.tile([128, T16, C], f32, tag="a_sb")
            a_red = sb.tile([128, T16, C], f32, tag="a_red")
            nc.vector.tensor_copy(out=a_sb[:], in_=a_psum[:])

            cc_in = dr.tile([128, T16 * C], f32, tag=f"cc_in{it}", name=f"cc_in{it}")
            cc_out = dr.tile([128, T16 * C], f32, tag=f"cc_out{it}",
                             name=f"cc_out{it}", addr_space="Shared")
            nc.sync.dma_start(out=cc_in[:], in_=a_sb[:])
            nc.gpsimd.collective_compute(
                "AllReduce", mybir.AluOpType.add, replica_groups=rg,
                ins=[cc_in[:].opt()], outs=[cc_out[:].opt()])
            nc.sync.dma_start(out=a_red[:], in_=cc_out[:])
            nc.vector.tensor_add(b_ij[:], b_ij[:], a_red[:])

    nc.finalize()
    return nc


_NC_CACHE = None


def kernel(u, W):
    """u [256,1152,8] f32, W [32,10,16,8] f32 -> [256,10,16,1] f32."""
    global _NC_CACHE
    from concourse import bass_utils

    in_maps = _host_prep(u, W)
    if _NC_CACHE is None:
        _NC_CACHE = _build_nc()
    res = bass_utils.run_bass_kernel_spmd(
        _NC_CACHE, in_maps, core_ids=list(range(NC_CORES)))

    out = np.zeros((NC_CORES * BL, C, O, 1), np.float32)
    for c in range(NC_CORES):
        vo = res.results[c]["v_out"]          # [16, C, BL] = [o, c, b]
        out[c * BL:(c + 1) * BL, :, :, 0] = vo.transpose(2, 1, 0)
    return out


if __name__ == "__main__":
    u = np.random.randn(256, 1152, 8).astype(np.float32)
    W = np.random.randn(32, 10, 16, 8).astype(np.float32)
    v = kernel(u, W)
    print("kernel ran, out shape", v.shape, "absmax", np.abs(v).max())
